# revision 1
# baseline (speedup 1.0000x reference)
"""Trainium2 Bass kernel for nn_ContrastiveLoss (SimCLR-style, N=8192, D=128).

Sharding: rows of the NxN sim matrix split across 8 cores (1024 rows each).
Each core receives the full z = concat(emb0, emb1) ROTATED so its own rows
come first (np.roll(z, -core*1024, axis=0)).  With that rotation the diagonal
of row-block b sits at local columns [b*128, b*128+128) and the positive pair
at local columns [4096+b*128, ...), identical on every core -> one SPMD
program, no collectives.

Math (per row r, fixed max = 1.0 since cosine sim <= 1):
  e_j  = exp(10*G_rj - 10),  S_r = sum_j e_j - e_rr
  loss_r = lse_r - 10*G_pos = (10 + ln S_r) - (ln e_pos + 10) = ln S_r - ln e_pos
  loss   = mean_r(loss_r);  per-core output = [128,1] partial sums of loss_r.

Engine split per core: PE does z_blk @ z^T (bf16 operands, fp32 psum)
plus the zn transposes; ACT does exp(10x-10) on each [128,2048] psum chunk
with accum_out row-sums; DVE does norms, psum->bf16 casts (batched 512 wide)
and diag/pos extraction from the exp output in SBUF.
"""

import sys

sys.path.insert(0, "/opt/trn_rl_repo")

from contextlib import ExitStack

import numpy as np

import concourse.bass as bass
import concourse.bacc as bacc
import concourse.tile as tile
from concourse import mybir
from concourse import bass_utils
from concourse.masks import make_identity

B = 4096
D = 128
N = 2 * B            # 8192 rows of z
NCORES = 8
ROWS = N // NCORES   # 1024 rows per core
NBLK = ROWS // 128   # 8 row-blocks per core
CHUNK = 2048         # psum tile width (4 banks)
NCHUNK = N // CHUNK  # 4 column chunks
SEG = 512            # matmul moving-operand width
NTILE = N // 128     # 64 partition-tiles of z
GRP = 8              # tiles per DMA / norm group
INV_T = 10.0         # 1/temperature
EPS = 1e-8

F32 = mybir.dt.float32
BF16 = mybir.dt.bfloat16
AX = mybir.AxisListType
AF = mybir.ActivationFunctionType


def _build() -> bass.Bass:
    nc = bacc.Bacc(None)
    z_in = nc.declare_dram_parameter("z", [N, D], F32, isOutput=False)
    out = nc.declare_dram_parameter("partial", [128, 1], F32, isOutput=True)

    z_re = z_in.rearrange("(n p) d -> p n d", p=128)  # row = n*128 + p

    with tile.TileContext(nc) as tc:
        with ExitStack() as ctx:
            persist = ctx.enter_context(tc.tile_pool(name="persist", bufs=1))
            work = ctx.enter_context(tc.tile_pool(name="work", bufs=3))
            junkp = ctx.enter_context(tc.tile_pool(name="junk", bufs=3))
            psum = ctx.enter_context(tc.tile_pool(name="psum", bufs=2, space="PSUM"))

            ident = persist.tile([128, 128], BF16)
            make_identity(nc, ident)
            # non-Copy activations need bias as an SBUF AP
            b_zero = persist.tile([128, 1], F32)
            nc.vector.memset(b_zero, 0.0)
            b_neg10 = persist.tile([128, 1], F32)
            nc.vector.memset(b_neg10, -INV_T)

            # ---- load z + per-group norms + normalize + transpose --------
            # Per 8-tile group: DMA -> sumsq -> rsqrt-norm -> bf16 zn ->
            # 8 PE transposes into one psum bank -> one 1024-wide cast.
            z_sb = persist.tile([128, NTILE, D], F32)
            sq = persist.tile([128, NTILE, D], F32)
            rn = persist.tile([128, NTILE], F32)
            zn_all = persist.tile([128, NTILE, D], BF16)
            znT = [
                persist.tile([128, CHUNK], BF16, tag=f"znT{j}", name=f"znT{j}")
                for j in range(NCHUNK)
            ]
            acc = persist.tile([128, NBLK, NCHUNK], F32)   # per-chunk exp sums
            e_diag = persist.tile([128, NBLK], F32)
            e_pos = persist.tile([128, NBLK], F32)

            # all input DMAs up front; the sync queue streams them back-to-back
            for i in range(NTILE // GRP):
                sl = slice(i * GRP, (i + 1) * GRP)
                nc.sync.dma_start(out=z_sb[:, sl, :], in_=z_re[:, sl, :])

            def norm_group(i):
                sl = slice(i * GRP, (i + 1) * GRP)
                nc.vector.tensor_mul(sq[:, sl, :], z_sb[:, sl, :], z_sb[:, sl, :])
                nc.vector.reduce_sum(rn[:, sl], sq[:, sl, :], axis=AX.X)
                nc.scalar.activation(rn[:, sl], rn[:, sl], AF.Sqrt, bias=b_zero)
                nc.vector.tensor_scalar_max(rn[:, sl], rn[:, sl], EPS)
                nc.vector.reciprocal(rn[:, sl], rn[:, sl])
                nc.vector.tensor_mul(
                    zn_all[:, sl, :],
                    z_sb[:, sl, :],
                    rn[:, sl].broadcast_to((128, GRP, D)),
                )
                tp = psum.tile([128, GRP * 128], BF16, tag="pp", name="tp")
                for q in range(GRP):
                    nc.tensor.transpose(
                        tp[:, q * 128 : (q + 1) * 128],
                        zn_all[:, i * GRP + q, :],
                        ident,
                    )
                j, k = divmod(i * GRP * 128, CHUNK)
                nc.vector.tensor_copy(znT[j][:, k : k + GRP * 128], tp)

            def emit_block(b, c):
                lhsT = znT[0][:, b * 128 : (b + 1) * 128]  # block cols < 1024
                pt = psum.tile([128, CHUNK], F32, tag="pp", name="pt")
                for s in range(CHUNK // SEG):
                    nc.tensor.matmul(
                        pt[:, s * SEG : (s + 1) * SEG],
                        lhsT,
                        znT[c][:, s * SEG : (s + 1) * SEG],
                        start=True,
                        stop=True,
                    )
                ej = junkp.tile([128, CHUNK], F32, tag="ej", name="ej")
                nc.scalar.activation(
                    ej, pt, AF.Exp, scale=INV_T, bias=b_neg10,
                    accum_out=acc[:, b, c : c + 1],
                )
                if c == 0:  # e_rr at cols b*128..+128 of chunk 0
                    scr = work.tile([128, 128], F32, tag="scr", name="scr")
                    nc.vector.tensor_mul(scr, ej[:, b * 128 : b * 128 + 128], ident)
                    nc.vector.reduce_sum(e_diag[:, b : b + 1], scr, axis=AX.X)
                if c == 2:  # e_pos at cols 4096 + b*128..+128
                    scr2 = work.tile([128, 128], F32, tag="scr2", name="scr2")
                    nc.vector.tensor_mul(scr2, ej[:, b * 128 : b * 128 + 128], ident)
                    nc.vector.reduce_sum(e_pos[:, b : b + 1], scr2, axis=AX.X)

            # Pass 0 interleaves the remaining norm groups PAIRWISE so the
            # 2-slot psum round-robin keeps consecutive pt tiles on opposite
            # slots (tp pairs between pt pairs); all znT chunks are ready
            # before pass 1 and the exp stream starts as soon as znT[0] is.
            norm_group(0); norm_group(1)
            emit_block(0, 0); emit_block(1, 0)
            norm_group(2); norm_group(3)
            emit_block(2, 0); emit_block(3, 0)
            norm_group(4); norm_group(5)
            emit_block(4, 0); emit_block(5, 0)
            norm_group(6); norm_group(7)
            emit_block(6, 0); emit_block(7, 0)
            for c in range(1, NCHUNK):
                for b in range(NBLK):
                    emit_block(b, c)

            # ---- epilogue ------------------------------------------------
            sumexp = persist.tile([128, NBLK], F32)
            nc.vector.reduce_sum(sumexp, acc, axis=AX.X)      # [128,8,4] -> [128,8]
            S = persist.tile([128, NBLK], F32)
            nc.vector.tensor_sub(S, sumexp, e_diag)
            lnS = persist.tile([128, NBLK], F32)
            nc.scalar.activation(lnS, S, AF.Ln, bias=b_zero)
            lnp = persist.tile([128, NBLK], F32)
            nc.scalar.activation(lnp, e_pos, AF.Ln, bias=b_zero)
            contrib = persist.tile([128, NBLK], F32)
            nc.vector.tensor_sub(contrib, lnS, lnp)
            total = persist.tile([128, 1], F32)
            nc.vector.reduce_sum(total, contrib, axis=AX.X)
            nc.sync.dma_start(out=out[:, :], in_=total)

    nc.compile()
    return nc


_NC = None


def _get_nc() -> bass.Bass:
    global _NC
    if _NC is None:
        _NC = _build()
    return _NC


def kernel(emb0: np.ndarray, emb1: np.ndarray) -> np.ndarray:
    z = np.concatenate(
        [np.asarray(emb0, np.float32), np.asarray(emb1, np.float32)], axis=0
    )
    in_maps = [
        {"z": np.ascontiguousarray(np.roll(z, -c * ROWS, axis=0))}
        for c in range(NCORES)
    ]
    res = bass_utils.run_bass_kernel_spmd(_get_nc(), in_maps, core_ids=list(range(NCORES)))
    total = sum(float(r["partial"].sum(dtype=np.float64)) for r in res.results)
    return np.asarray(np.float32(total / N))



# revision 9
# speedup vs baseline: 1.0912x; 1.0912x over previous
"""Trainium2 Bass kernel for nn_ContrastiveLoss (SimCLR-style, N=8192, D=128).

v3: exploits symmetry of the sim matrix.  Global rows are split into 64
blocks of 128; core c owns blocks 8c..8c+7 (rows rotated so they are local
blocks 0..7).  For local row-block b the kernel computes only the wrap-band
of blocks (b, b+d) for d = 0..32 (local col-blocks b..b+32, never wrapping
since b<=7).  Row sums over d=0..31 come free from ACT accum_out on the exp;
the d=32 "tail" blocks are batched and row-summed on DVE.  The transposed
contributions (offsets 33..63 of each row) are obtained from column sums of
the d=1..31 blocks: ones[128,32]^T @ E matmuls accumulate per-column sums
into a packed PSUM region (seg s of 512 cols lives at partition-group
32*(s%4), one bank per 4 segs, via tile_position col-tiling).  Per-core
outputs are tiny ([128,24] row partials + [128,1536] col partials); the host
scatters column sums to their owning global rows, adds row partials,
subtracts the diagonal exp(0)=1, and finishes with log + mean.  Host also
pre-normalizes z (O(N*D)) so the device only does the O(N^2*D) work.

Engine budget per core: ACT 17 exp instructions (16 strip pairs of 2048 +
one 1024 tail batch) ~= 38us; PE 40 transposes + 33792 strip-matmul cols +
28160 colsum cols; DVE casts, psum->SBUF copies, tail reduce, CS drains.
PSUM: PT [128,3,1024] f32 (6 banks, rotating 1024-col chunk slots, also
reused by phase-1 transposes) + CS [128,1024] f32 (2 banks, 8 colsum seg
slots; bank 6 is drained mid-kernel and recycled for segs 8-9).
"""

import sys

sys.path.insert(0, "/opt/trn_rl_repo")

from contextlib import ExitStack

import numpy as np

import concourse.bass as bass
import concourse.bacc as bacc
import concourse.tile as tile
from concourse import mybir
from concourse import bass_utils
from concourse.masks import make_identity

B = 4096
D = 128
N = 2 * B            # 8192 rows of z
NCORES = 8
ROWS = N // NCORES   # 1024 rows per core
NBLK = 8             # strips (row blocks) per core
NT = 40              # znT col-blocks needed per core (local blocks 0..39)
ZROWS = NT * 128     # 5120 rows of rotated zn shipped per core
SEG = 512
CHUNK = 1024         # PT slot width
INV_T = 10.0
EPS = 1e-8

F32 = mybir.dt.float32
BF16 = mybir.dt.bfloat16
AX = mybir.AxisListType
AF = mybir.ActivationFunctionType


def _cs_slot(seg, life2=False):
    """Colsum psum placement: seg (512 abs cols) -> (bank_col, grp)."""
    if life2:  # segs 8, 9 reuse bank 0 after the mid-kernel drain
        return 0, seg - 8
    if seg < 4:
        return 0, seg
    return 512, seg - 4


def _build() -> bass.Bass:
    nc = bacc.Bacc(None)
    z_in = nc.declare_dram_parameter("z", [ZROWS, D], F32, isOutput=False)
    out_acc = nc.declare_dram_parameter("acc", [128, 3 * NBLK], F32, isOutput=True)
    out_cs = nc.declare_dram_parameter("cs", [128, 1536], F32, isOutput=True)

    z_re = z_in.rearrange("(n p) d -> p n d", p=128)  # local row = n*128 + p

    with tile.TileContext(nc) as tc:
        with ExitStack() as ctx:
            persist = ctx.enter_context(tc.tile_pool(name="persist", bufs=1))
            epool = ctx.enter_context(tc.tile_pool(name="epool", bufs=3))
            psA = ctx.enter_context(tc.tile_pool(name="psA", bufs=1, space="PSUM"))
            psB = ctx.enter_context(tc.tile_pool(name="psB", bufs=1, space="PSUM"))

            ident = persist.tile([128, 128], F32)
            make_identity(nc, ident)
            ones32 = persist.tile([128, 32], BF16)
            nc.vector.memset(ones32, 1.0)
            b_neg10 = persist.tile([128, 1], F32)
            nc.vector.memset(b_neg10, -INV_T)

            z_sb = persist.tile([128, NT, D], F32)
            znT = persist.tile([128, ZROWS], BF16)
            acc_sb = persist.tile([128, 3 * NBLK], F32)
            cs_sb = persist.tile([128, 1536], F32)

            PT = psA.tile([128, 3, CHUNK], F32)   # 6 banks
            CS = psB.tile([128, 1024], F32)       # 2 banks

            # ---- slot ring over PT ----------------------------------------
            state = {"cnt": 0}

            def next_slot():
                s = state["cnt"] % 3
                state["cnt"] += 1
                return s

            # ---- phase 1: load + cast + transpose -------------------------
            GRP = 8
            for g in range(NT // GRP):
                sl = slice(g * GRP, (g + 1) * GRP)
                nc.sync.dma_start(out=z_sb[:, sl, :], in_=z_re[:, sl, :])

            def do_group(g):
                slot = next_slot()
                for k in range(GRP):
                    nc.tensor.transpose(
                        PT[:, slot, k * 128 : (k + 1) * 128],
                        z_sb[:, g * GRP + k, :],
                        ident,
                    )
                nc.vector.tensor_copy(
                    znT[:, g * GRP * 128 : (g + 1) * GRP * 128],
                    PT[:, slot, :],
                )

            # ---- colsum bookkeeping ---------------------------------------
            # CS banks are zeroed by DVE memset; all colsum matmuls then use
            # start=False: stale has_written bits make the mm accumulate onto
            # the zeroed value, cleared bits make it overwrite -- both correct.
            nc.vector.memset(CS, 0.0)
            cs_life2 = {"on": False}

            # statically compute the last colsum mm per bank-life for stop=True
            # colsum mms exist for (b, d) with d in 1..31; emission order is
            # supers dp=0 b=0..7 then dp=1 b=0..7; within a super, d ascending.
            mm_seq = []
            for dp in range(2):
                for b in range(NBLK):
                    for d in range(max(1, 16 * dp), 16 * dp + 16):
                        if 1 <= d <= 31:
                            mm_seq.append((b, dp, d))
            last_of = {}
            for idx, (b, dp, d) in enumerate(mm_seq):
                jb = b + d
                seg = jb // 4
                life2 = seg >= 8
                bank = _cs_slot(seg, life2)[0]
                last_of[(bank, life2)] = (b, dp, d)

            def colsum_mms(b, dp, e_pos_of_dc, E):
                """Emit colsum matmuls for super (b, dp) consuming E."""
                d_lo = max(1, 16 * dp)
                d_hi = 16 * dp + 16
                # group consecutive d with same (seg, chunk, flags) into runs
                d = d_lo
                while d < d_hi:
                    if not (1 <= d <= 31):
                        d += 1
                        continue
                    jb = b + d
                    seg = jb // 4
                    life2 = seg >= 8
                    dc = d // 8
                    bank, grp = _cs_slot(seg, life2)
                    if life2 and not cs_life2["on"]:
                        raise RuntimeError("life2 before drain")
                    run = 1
                    while (
                        d + run < d_hi
                        and d + run <= 31
                        and (b + d + run) // 4 == seg
                        and (d + run) // 8 == dc
                    ):
                        run += 1
                    stop = False
                    for dd in range(d, d + run):
                        if last_of.get((bank, life2)) == (b, dp, dd):
                            stop = True
                    epos = e_pos_of_dc[dc]
                    k = d % 8
                    off = (jb * 128) % 512
                    nc.tensor.matmul(
                        CS[32 * grp : 32 * grp + 32, bank + off : bank + off + run * 128],
                        ones32,
                        E[:, epos, k * 128 : (k + run) * 128],
                        start=False,
                        stop=stop,
                        tile_position=(0, 32 * grp),
                        skip_group_check=True,
                    )
                    d += run

            # ---- super unit: two 1024 chunks + paired exp -----------------
            def do_super(b, dp):
                slots = []
                for q in range(2):
                    dc = 2 * dp + q
                    slot = next_slot()
                    slots.append(slot)
                    c0 = b * 128 + dc * CHUNK
                    for s in range(CHUNK // SEG):
                        nc.tensor.matmul(
                            PT[:, slot, s * SEG : (s + 1) * SEG],
                            znT[:, b * 128 : (b + 1) * 128],
                            znT[:, c0 + s * SEG : c0 + (s + 1) * SEG],
                            start=True,
                            stop=True,
                        )
                s0, s1 = slots
                if (s0, s1) in ((0, 1), (1, 2)):
                    in_ap = PT[:, s0 : s1 + 1, :]
                    e_pos_of_dc = {2 * dp: 0, 2 * dp + 1: 1}
                elif (s0, s1) == (2, 0):
                    in_ap = PT[:, 0:3:2, :]  # [slot0, slot2] = [q1, q0]
                    e_pos_of_dc = {2 * dp: 1, 2 * dp + 1: 0}
                else:
                    raise RuntimeError(f"bad slot pair {s0},{s1}")
                E = epool.tile([128, 2, CHUNK], BF16, tag="E", name="E")
                nc.scalar.activation(
                    E, in_ap, AF.Exp, scale=INV_T, bias=b_neg10,
                    accum_out=acc_sb[:, 2 * b + dp : 2 * b + dp + 1],
                )
                colsum_mms(b, dp, e_pos_of_dc, E)

            # ---- emission order -------------------------------------------
            do_group(0)
            do_group(1)
            do_group(2)
            for b in range(NBLK):
                do_super(b, 0)
                if b == 0:
                    do_group(3)
                elif b == 2:
                    do_group(4)
            do_super(0, 1)
            # drain bank 6 (segs 0..3) and recycle it for segs 8..9
            nc.vector.tensor_copy(cs_sb[:, 0:512], CS[:, 0:512])
            nc.vector.memset(CS[:, 0:512], 0.0)
            cs_life2["on"] = True
            for b in range(1, NBLK):
                do_super(b, 1)

            # ---- tails: blocks (b, b+32), batched -------------------------
            slot = next_slot()
            for b in range(NBLK):
                nc.tensor.matmul(
                    PT[:, slot, b * 128 : (b + 1) * 128],
                    znT[:, b * 128 : (b + 1) * 128],
                    znT[:, (b + 32) * 128 : (b + 33) * 128],
                    start=True,
                    stop=True,
                )
            Et = epool.tile([128, 2, CHUNK], BF16, tag="E", name="Etail")
            nc.scalar.activation(
                Et[:, 0, :], PT[:, slot, :], AF.Exp, scale=INV_T, bias=b_neg10,
            )
            nc.vector.reduce_sum(
                acc_sb[:, 2 * NBLK : 3 * NBLK],
                Et[:, 0, :].rearrange("p (a b) -> p a b", a=NBLK),
                axis=AX.X,
            )

            # ---- final drains + output ------------------------------------
            nc.vector.tensor_copy(cs_sb[:, 512:1536], CS[:, :])
            nc.sync.dma_start(out=out_acc[:, :], in_=acc_sb)
            nc.sync.dma_start(out=out_cs[:, :], in_=cs_sb)

    nc.compile()
    return nc


_NC = None


def _get_nc() -> bass.Bass:
    global _NC
    if _NC is None:
        _NC = _build()
    return _NC


def prepare_in_maps(emb0: np.ndarray, emb1: np.ndarray):
    z = np.concatenate(
        [np.asarray(emb0, np.float32), np.asarray(emb1, np.float32)], axis=0
    )
    nrm = np.maximum(np.linalg.norm(z, axis=1, keepdims=True), EPS)
    zn = (z / nrm).astype(np.float32)
    in_maps = [
        {"z": np.ascontiguousarray(np.roll(zn, -c * ROWS, axis=0)[:ZROWS])}
        for c in range(NCORES)
    ]
    return zn, in_maps


def combine(zn: np.ndarray, results) -> np.ndarray:
    S = np.zeros(N, dtype=np.float64)
    for c in range(NCORES):
        acc = np.asarray(results[c]["acc"], np.float64)   # [128, 24]
        cs = np.asarray(results[c]["cs"], np.float64)     # [128, 1536]
        # row partial sums: strips' pair accums + tail sums - diag
        for b in range(NBLK):
            rows = (c * ROWS + b * 128 + np.arange(128)) % N
            S[rows] += acc[:, 2 * b] + acc[:, 2 * b + 1] + acc[:, 2 * NBLK + b] - 1.0
        # column partial sums: abs cols 128..4991 -> global rows
        j = np.arange(128, 4992)
        seg = j // 512
        off = j % 512
        vals = np.empty(j.shape, np.float64)
        for s in range(10):
            m = seg == s
            if s < 4:
                vals[m] = cs[32 * s, off[m]]                  # drain1: bank6 life1
            elif s < 8:
                vals[m] = cs[32 * (s - 4), 1024 + off[m]]     # final: bank7
            else:
                vals[m] = cs[32 * (s - 8), 512 + off[m]]      # final: bank6 life2
        rows = (c * ROWS + j) % N
        np.add.at(S, rows, vals)
    pos = (zn * np.roll(zn, -B, axis=0)).sum(axis=1)  # cos of positive pair
    loss = np.log(S) + INV_T - INV_T * pos
    return np.asarray(np.float32(loss.mean()))


def kernel(emb0: np.ndarray, emb1: np.ndarray) -> np.ndarray:
    zn, in_maps = prepare_in_maps(emb0, emb1)
    res = bass_utils.run_bass_kernel_spmd(_get_nc(), in_maps, core_ids=list(range(NCORES)))
    return combine(zn, res.results)


# revision 11
# speedup vs baseline: 1.1606x; 1.0636x over previous
"""Trainium2 Bass kernel for nn_ContrastiveLoss (SimCLR-style, N=8192, D=128).

v3: exploits symmetry of the sim matrix.  Global rows are split into 64
blocks of 128; core c owns blocks 8c..8c+7 (rows rotated so they are local
blocks 0..7).  For local row-block b the kernel computes only the wrap-band
of blocks (b, b+d) for d = 0..32 (local col-blocks b..b+32, never wrapping
since b<=7).  Row sums over d=0..31 come free from ACT accum_out on the exp;
the d=32 "tail" blocks are batched and row-summed on DVE.  The transposed
contributions (offsets 33..63 of each row) are obtained from column sums of
the d=1..31 blocks: ones[128,32]^T @ E matmuls accumulate per-column sums
into a packed PSUM region (seg s of 512 cols lives at partition-group
32*(s%4), one bank per 4 segs, via tile_position col-tiling).  Per-core
outputs are tiny ([128,24] row partials + [128,1536] col partials); the host
scatters column sums to their owning global rows, adds row partials,
subtracts the diagonal exp(0)=1, and finishes with log + mean.  Host also
pre-normalizes z (O(N*D)) so the device only does the O(N^2*D) work.

Engine budget per core: ACT 17 exp instructions (16 strip pairs of 2048 +
one 1024 tail batch) ~= 38us; PE 40 transposes + 33792 strip-matmul cols +
28160 colsum cols; DVE casts, psum->SBUF copies, tail reduce, CS drains.
PSUM: PT [128,3,1024] f32 (6 banks, rotating 1024-col chunk slots, also
reused by phase-1 transposes) + CS [128,1024] f32 (2 banks, 8 colsum seg
slots; bank 6 is drained mid-kernel and recycled for segs 8-9).
"""

import sys

sys.path.insert(0, "/opt/trn_rl_repo")

from contextlib import ExitStack

import numpy as np

import concourse.bass as bass
import concourse.bacc as bacc
import concourse.tile as tile
from concourse import mybir
from concourse import bass_utils
from concourse.masks import make_identity

B = 4096
D = 128
N = 2 * B            # 8192 rows of z
NCORES = 8
ROWS = N // NCORES   # 1024 rows per core
NBLK = 8             # strips (row blocks) per core
NT = 40              # znT col-blocks needed per core (local blocks 0..39)
ZROWS = NT * 128     # 5120 rows of rotated zn shipped per core
SEG = 512
CHUNK = 1024         # PT slot width
INV_T = 10.0
EPS = 1e-8

F32 = mybir.dt.float32
BF16 = mybir.dt.bfloat16
AX = mybir.AxisListType
AF = mybir.ActivationFunctionType


def _cs_slot(seg, life2=False):
    """Colsum psum placement: seg (512 abs cols) -> (bank_col, grp)."""
    if life2:  # segs 8, 9 reuse bank 0 after the mid-kernel drain
        return 0, seg - 8
    if seg < 4:
        return 0, seg
    return 512, seg - 4


def _build() -> bass.Bass:
    nc = bacc.Bacc(None)
    z_in = nc.declare_dram_parameter("z", [ZROWS, D], F32, isOutput=False)
    out_acc = nc.declare_dram_parameter("acc", [128, 3 * NBLK], F32, isOutput=True)
    out_cs = nc.declare_dram_parameter("cs", [128, 1536], F32, isOutput=True)

    z_re = z_in.rearrange("(n p) d -> p n d", p=128)  # local row = n*128 + p

    with tile.TileContext(nc) as tc:
        with ExitStack() as ctx:
            persist = ctx.enter_context(tc.tile_pool(name="persist", bufs=1))
            epool = ctx.enter_context(tc.tile_pool(name="epool", bufs=3))
            psA = ctx.enter_context(tc.tile_pool(name="psA", bufs=1, space="PSUM"))
            psB = ctx.enter_context(tc.tile_pool(name="psB", bufs=1, space="PSUM"))

            ident = persist.tile([128, 128], F32)
            make_identity(nc, ident)
            ones32 = persist.tile([128, 32], BF16)
            nc.vector.memset(ones32, 1.0)
            b_neg10 = persist.tile([128, 1], F32)
            nc.vector.memset(b_neg10, -INV_T)

            z_sb = persist.tile([128, NT, D], F32)
            znT = persist.tile([128, ZROWS], BF16)
            acc_sb = persist.tile([128, 3 * NBLK], F32)
            cs_sb = persist.tile([128, 1536], F32)

            PT = psA.tile([128, 3, CHUNK], F32)   # 6 banks
            CS = psB.tile([128, 1024], F32)       # 2 banks

            # ---- slot ring over PT ----------------------------------------
            state = {"cnt": 0}

            def next_slot():
                s = state["cnt"] % 3
                state["cnt"] += 1
                return s

            # ---- phase 1: load + cast + transpose -------------------------
            GRP = 8
            for g in range(NT // GRP):
                sl = slice(g * GRP, (g + 1) * GRP)
                nc.sync.dma_start(out=z_sb[:, sl, :], in_=z_re[:, sl, :])

            def do_group(g):
                slot = next_slot()
                for k in range(GRP):
                    nc.tensor.transpose(
                        PT[:, slot, k * 128 : (k + 1) * 128],
                        z_sb[:, g * GRP + k, :],
                        ident,
                    )
                nc.vector.tensor_copy(
                    znT[:, g * GRP * 128 : (g + 1) * GRP * 128],
                    PT[:, slot, :],
                )

            # ---- colsum bookkeeping ---------------------------------------
            # CS banks are zeroed by DVE memset; all colsum matmuls then use
            # start=False: stale has_written bits make the mm accumulate onto
            # the zeroed value, cleared bits make it overwrite -- both correct.
            nc.vector.memset(CS, 0.0)
            cs_life2 = {"on": False}

            # statically compute the last colsum mm per bank-life for stop=True
            # colsum mms exist for (b, d) with d in 1..31; emission order is
            # supers dp=0 b=0..7 then dp=1 b=0..7; within a super, d ascending.
            mm_seq = []
            for dp in range(2):
                for b in range(NBLK):
                    for d in range(max(1, 16 * dp), 16 * dp + 16):
                        if 1 <= d <= 31:
                            mm_seq.append((b, dp, d))
            last_of = {}
            for idx, (b, dp, d) in enumerate(mm_seq):
                jb = b + d
                seg = jb // 4
                life2 = seg >= 8
                bank = _cs_slot(seg, life2)[0]
                last_of[(bank, life2)] = (b, dp, d)

            def colsum_mms(b, dp, e_pos_of_dc, E):
                """Emit colsum matmuls for super (b, dp) consuming E."""
                d_lo = max(1, 16 * dp)
                d_hi = 16 * dp + 16
                # group consecutive d with same (seg, chunk, flags) into runs
                d = d_lo
                while d < d_hi:
                    if not (1 <= d <= 31):
                        d += 1
                        continue
                    jb = b + d
                    seg = jb // 4
                    life2 = seg >= 8
                    dc = d // 8
                    bank, grp = _cs_slot(seg, life2)
                    if life2 and not cs_life2["on"]:
                        raise RuntimeError("life2 before drain")
                    run = 1
                    while (
                        d + run < d_hi
                        and d + run <= 31
                        and (b + d + run) // 4 == seg
                        and (d + run) // 8 == dc
                    ):
                        run += 1
                    stop = False
                    for dd in range(d, d + run):
                        if last_of.get((bank, life2)) == (b, dp, dd):
                            stop = True
                    epos = e_pos_of_dc[dc]
                    k = d % 8
                    off = (jb * 128) % 512
                    nc.tensor.matmul(
                        CS[32 * grp : 32 * grp + 32, bank + off : bank + off + run * 128],
                        ones32,
                        E[:, epos, k * 128 : (k + run) * 128],
                        start=False,
                        stop=stop,
                        tile_position=(0, 32 * grp),
                        skip_group_check=True,
                    )
                    d += run

            # ---- super unit: two 1024 chunks + paired exp -----------------
            def do_fills(b, dp):
                slots = []
                for q in range(2):
                    dc = 2 * dp + q
                    slot = next_slot()
                    slots.append(slot)
                    c0 = b * 128 + dc * CHUNK
                    for s in range(CHUNK // SEG):
                        nc.tensor.matmul(
                            PT[:, slot, s * SEG : (s + 1) * SEG],
                            znT[:, b * 128 : (b + 1) * 128],
                            znT[:, c0 + s * SEG : c0 + (s + 1) * SEG],
                            start=True,
                            stop=True,
                        )
                return tuple(slots)

            def do_act(b, dp, slots):
                s0, s1 = slots
                if (s0, s1) in ((0, 1), (1, 2)):
                    in_ap = PT[:, s0 : s1 + 1, :]
                    e_pos_of_dc = {2 * dp: 0, 2 * dp + 1: 1}
                elif (s0, s1) == (2, 0):
                    in_ap = PT[:, 0:3:2, :]  # [slot0, slot2] = [q1, q0]
                    e_pos_of_dc = {2 * dp: 1, 2 * dp + 1: 0}
                else:
                    raise RuntimeError(f"bad slot pair {s0},{s1}")
                E = epool.tile([128, 2, CHUNK], BF16, tag="E", name="E")
                nc.scalar.activation(
                    E, in_ap, AF.Exp, scale=INV_T, bias=b_neg10,
                    accum_out=acc_sb[:, 2 * b + dp : 2 * b + dp + 1],
                )
                return E, e_pos_of_dc

            # ---- emission order: colsums of super k go to the PE queue ----
            # AFTER the fills of super k+1, so the PE never waits on ACT's E
            # while independent fill work exists (engine queues are in-order).
            supers = [(b, 0) for b in range(NBLK)] + [(b, 1) for b in range(NBLK)]
            do_group(0)
            do_group(1)
            do_group(2)
            pend = None
            for k, (b, dp) in enumerate(supers):
                slots = do_fills(b, dp)
                if pend is not None:
                    colsum_mms(*pend)
                if (b, dp) == (1, 1):
                    # drain bank 6 (segs 0..3) and recycle it for segs 8..9;
                    # emitted before colsums(1,1) which first touches seg 8
                    nc.vector.tensor_copy(cs_sb[:, 0:512], CS[:, 0:512])
                    nc.vector.memset(CS[:, 0:512], 0.0)
                    cs_life2["on"] = True
                E, epos = do_act(b, dp, slots)
                pend = (b, dp, epos, E)
                if (b, dp) == (0, 0):
                    do_group(3)
                elif (b, dp) == (2, 0):
                    do_group(4)

            # ---- tails: blocks (b, b+32), batched -------------------------
            slot = next_slot()
            for b in range(NBLK):
                nc.tensor.matmul(
                    PT[:, slot, b * 128 : (b + 1) * 128],
                    znT[:, b * 128 : (b + 1) * 128],
                    znT[:, (b + 32) * 128 : (b + 33) * 128],
                    start=True,
                    stop=True,
                )
            colsum_mms(*pend)
            Et = epool.tile([128, 2, CHUNK], BF16, tag="E", name="Etail")
            nc.scalar.activation(
                Et[:, 0, :], PT[:, slot, :], AF.Exp, scale=INV_T, bias=b_neg10,
            )
            nc.vector.reduce_sum(
                acc_sb[:, 2 * NBLK : 3 * NBLK],
                Et[:, 0, :].rearrange("p (a b) -> p a b", a=NBLK),
                axis=AX.X,
            )

            # ---- final drains + output ------------------------------------
            nc.vector.tensor_copy(cs_sb[:, 512:1536], CS[:, :])
            nc.sync.dma_start(out=out_acc[:, :], in_=acc_sb)
            nc.sync.dma_start(out=out_cs[:, :], in_=cs_sb)

    nc.compile()
    return nc


_NC = None


def _get_nc() -> bass.Bass:
    global _NC
    if _NC is None:
        _NC = _build()
    return _NC


def prepare_in_maps(emb0: np.ndarray, emb1: np.ndarray):
    z = np.concatenate(
        [np.asarray(emb0, np.float32), np.asarray(emb1, np.float32)], axis=0
    )
    nrm = np.maximum(np.linalg.norm(z, axis=1, keepdims=True), EPS)
    zn = (z / nrm).astype(np.float32)
    in_maps = [
        {"z": np.ascontiguousarray(np.roll(zn, -c * ROWS, axis=0)[:ZROWS])}
        for c in range(NCORES)
    ]
    return zn, in_maps


def combine(zn: np.ndarray, results) -> np.ndarray:
    S = np.zeros(N, dtype=np.float64)
    for c in range(NCORES):
        acc = np.asarray(results[c]["acc"], np.float64)   # [128, 24]
        cs = np.asarray(results[c]["cs"], np.float64)     # [128, 1536]
        # row partial sums: strips' pair accums + tail sums - diag
        for b in range(NBLK):
            rows = (c * ROWS + b * 128 + np.arange(128)) % N
            S[rows] += acc[:, 2 * b] + acc[:, 2 * b + 1] + acc[:, 2 * NBLK + b] - 1.0
        # column partial sums: abs cols 128..4991 -> global rows
        j = np.arange(128, 4992)
        seg = j // 512
        off = j % 512
        vals = np.empty(j.shape, np.float64)
        for s in range(10):
            m = seg == s
            if s < 4:
                vals[m] = cs[32 * s, off[m]]                  # drain1: bank6 life1
            elif s < 8:
                vals[m] = cs[32 * (s - 4), 1024 + off[m]]     # final: bank7
            else:
                vals[m] = cs[32 * (s - 8), 512 + off[m]]      # final: bank6 life2
        rows = (c * ROWS + j) % N
        np.add.at(S, rows, vals)
    pos = (zn * np.roll(zn, -B, axis=0)).sum(axis=1)  # cos of positive pair
    loss = np.log(S) + INV_T - INV_T * pos
    return np.asarray(np.float32(loss.mean()))


def kernel(emb0: np.ndarray, emb1: np.ndarray) -> np.ndarray:
    zn, in_maps = prepare_in_maps(emb0, emb1)
    res = bass_utils.run_bass_kernel_spmd(_get_nc(), in_maps, core_ids=list(range(NCORES)))
    return combine(zn, res.results)


# revision 13
# speedup vs baseline: 2.0797x; 1.7920x over previous
"""Trainium2 Bass kernel for nn_ContrastiveLoss (SimCLR-style, N=8192, D=128).

v4: symmetry + host layout prep.  Global rows form 64 blocks of 128; core c
owns blocks 8c..8c+7.  The host normalizes z, rotates it per core, and ships
the TRANSPOSED bf16 matrix znT [128, 5120] (local col-blocks 0..39), so the
device does zero layout work.  Per local row-block b (strip), the device
computes sim blocks at offsets d=0..32 only (half the matrix, wrap-band):
three chunks of 1536/1536/1152 columns, each matmul'd into a rotating PSUM
tile and exponentiated by ACT (exp(10x-10), bf16 out, fp32 accum_out gives
the row sums).  Transposed contributions (offsets 33..63 of each row) come
from column sums of the d=1..31 blocks: ones[128,32]^T @ E matmuls accumulate
into a packed CS PSUM region (seg s of 512 abs cols -> partition-group slot
via tile_position; bank 6 is drained mid-kernel and recycled for segs 8-9).
Colsums of chunk k are emitted after the fills of chunk k+1 so the in-order
PE queue never stalls on ACT.  Host: scatter colsums to owning rows, add row
sums, subtract diag exp(0)=1, then loss = mean(ln S + 10 - 10*cos_pos).
"""

import sys

sys.path.insert(0, "/opt/trn_rl_repo")

from contextlib import ExitStack

import numpy as np

import concourse.bass as bass
import concourse.bacc as bacc
import concourse.tile as tile
from concourse import mybir
from concourse import bass_utils

B = 4096
D = 128
N = 2 * B
NCORES = 8
ROWS = N // NCORES   # 1024 rows per core
NBLK = 8             # strips per core
NT = 40              # znT col-blocks per core (local blocks 0..39)
ZCOLS = NT * 128     # 5120
SEG = 512
CHUNK = 1536         # chunk 0/1 width; chunk 2 is 1152
INV_T = 10.0
EPS = 1e-8

F32 = mybir.dt.float32
BF16 = mybir.dt.bfloat16
AX = mybir.AxisListType
AF = mybir.ActivationFunctionType


def _cs_slot(seg, life2=False):
    """Colsum psum placement: seg (512 abs cols) -> (bank_col, grp)."""
    if life2:  # segs 8, 9 reuse bank col 0 after the mid-kernel drain
        return 0, seg - 8
    if seg < 4:
        return 0, seg
    return 512, seg - 4


def _chunk_w(dc):
    return CHUNK if dc < 2 else 1152


def _build() -> bass.Bass:
    nc = bacc.Bacc(None)
    zT_in = nc.declare_dram_parameter("zT", [128, ZCOLS], BF16, isOutput=False)
    out_acc = nc.declare_dram_parameter("acc", [128, 3 * NBLK], F32, isOutput=True)
    out_cs = nc.declare_dram_parameter("cs", [128, 1536], F32, isOutput=True)

    with tile.TileContext(nc) as tc:
        with ExitStack() as ctx:
            persist = ctx.enter_context(tc.tile_pool(name="persist", bufs=1))
            epool = ctx.enter_context(tc.tile_pool(name="epool", bufs=3))
            ptpool = ctx.enter_context(tc.tile_pool(name="ptpool", bufs=2, space="PSUM"))
            psB = ctx.enter_context(tc.tile_pool(name="psB", bufs=1, space="PSUM"))

            ones32 = persist.tile([128, 32], BF16)
            nc.vector.memset(ones32, 1.0)
            b_neg10 = persist.tile([128, 1], F32)
            nc.vector.memset(b_neg10, -INV_T)

            znT = persist.tile([128, ZCOLS], BF16)
            acc_sb = persist.tile([128, 3 * NBLK], F32)
            cs_sb = persist.tile([128, 1536], F32)

            CS = psB.tile([128, 1024], F32)       # 2 banks
            nc.vector.memset(CS, 0.0)

            # input DMA in two pieces so compute can start early
            half = ZCOLS // 2
            nc.sync.dma_start(out=znT[:, 0:half], in_=zT_in[:, 0:half])
            nc.sync.dma_start(out=znT[:, half:ZCOLS], in_=zT_in[:, half:ZCOLS])

            cs_life2 = {"on": False}

            # static last-writer per CS bank-life for stop flags
            chunk_order = [(dc, b) for dc in range(3) for b in range(NBLK)]
            last_of = {}
            for (dc, b) in chunk_order:
                for d in range(max(1, 12 * dc), min(32, 12 * dc + 12)):
                    jb = b + d
                    seg = jb // 4
                    bank = _cs_slot(seg, seg >= 8)[0]
                    last_of[(bank, seg >= 8)] = (dc, b, d)

            def colsum_mms(dc, b, E):
                d = max(1, 12 * dc)
                d_hi = min(32, 12 * dc + 12)
                while d < d_hi:
                    jb = b + d
                    seg = jb // 4
                    life2 = seg >= 8
                    if life2 and not cs_life2["on"]:
                        raise RuntimeError("life2 before drain")
                    bank, grp = _cs_slot(seg, life2)
                    run = 1
                    while (
                        d + run < d_hi
                        and (b + d + run) // 4 == seg
                    ):
                        run += 1
                    stop = any(
                        last_of.get((bank, life2)) == (dc, b, dd)
                        for dd in range(d, d + run)
                    )
                    k = d - 12 * dc
                    off = (jb * 128) % 512
                    nc.tensor.matmul(
                        CS[32 * grp : 32 * grp + 32, bank + off : bank + off + run * 128],
                        ones32,
                        E[:, k * 128 : (k + run) * 128],
                        start=False,
                        stop=stop,
                        tile_position=(0, 32 * grp),
                        skip_group_check=True,
                    )
                    d += run

            pend = None
            for (dc, b) in chunk_order:
                W = _chunk_w(dc)
                pt = ptpool.tile([128, CHUNK], F32, tag="pt", name="pt")
                c0 = b * 128 + dc * CHUNK
                off = 0
                while off < W:
                    w = min(SEG, W - off)
                    nc.tensor.matmul(
                        pt[:, off : off + w],
                        znT[:, b * 128 : (b + 1) * 128],
                        znT[:, c0 + off : c0 + off + w],
                        start=True,
                        stop=True,
                    )
                    off += w
                if pend is not None:
                    if pend[0][:2] == (2, 1):
                        # drain bank 6 (segs 0..3), recycle for segs 8..9;
                        # colsums of chunk (2,1) write seg 8 first
                        nc.vector.tensor_copy(cs_sb[:, 0:512], CS[:, 0:512])
                        nc.vector.memset(CS[:, 0:512], 0.0)
                        cs_life2["on"] = True
                    colsum_mms(*pend[0][:2], pend[1])
                E = epool.tile([128, CHUNK], BF16, tag="E", name="E")
                nc.scalar.activation(
                    E[:, 0:W], pt[:, 0:W], AF.Exp, scale=INV_T, bias=b_neg10,
                    accum_out=acc_sb[:, 3 * b + dc : 3 * b + dc + 1],
                )
                pend = ((dc, b), E)
            if pend[0][:2] == (2, 1):
                raise RuntimeError("unexpected")
            colsum_mms(*pend[0][:2], pend[1])

            nc.vector.tensor_copy(cs_sb[:, 512:1536], CS[:, :])
            nc.sync.dma_start(out=out_acc[:, :], in_=acc_sb)
            nc.sync.dma_start(out=out_cs[:, :], in_=cs_sb)

    nc.compile()
    return nc


_NC = None


def _get_nc() -> bass.Bass:
    global _NC
    if _NC is None:
        _NC = _build()
    return _NC


def prepare_in_maps(emb0: np.ndarray, emb1: np.ndarray):
    import ml_dtypes

    z = np.concatenate(
        [np.asarray(emb0, np.float32), np.asarray(emb1, np.float32)], axis=0
    )
    nrm = np.maximum(np.linalg.norm(z, axis=1, keepdims=True), EPS)
    zn = (z / nrm).astype(np.float32)
    in_maps = []
    for c in range(NCORES):
        zr = np.roll(zn, -c * ROWS, axis=0)[:ZCOLS]
        zT = np.ascontiguousarray(zr.T).astype(ml_dtypes.bfloat16)
        in_maps.append({"zT": zT})
    return zn, in_maps


def combine(zn: np.ndarray, results) -> np.ndarray:
    S = np.zeros(N, dtype=np.float64)
    for c in range(NCORES):
        acc = np.asarray(results[c]["acc"], np.float64)   # [128, 24]
        cs = np.asarray(results[c]["cs"], np.float64)     # [128, 1536]
        for b in range(NBLK):
            rows = (c * ROWS + b * 128 + np.arange(128)) % N
            S[rows] += acc[:, 3 * b] + acc[:, 3 * b + 1] + acc[:, 3 * b + 2] - 1.0
        j = np.arange(128, 4992)
        seg = j // 512
        off = j % 512
        vals = np.empty(j.shape, np.float64)
        for s in range(10):
            m = seg == s
            if s < 4:
                vals[m] = cs[32 * s, off[m]]                  # drain1: bank6 life1
            elif s < 8:
                vals[m] = cs[32 * (s - 4), 1024 + off[m]]     # final: bank7
            else:
                vals[m] = cs[32 * (s - 8), 512 + off[m]]      # final: bank6 life2
        rows = (c * ROWS + j) % N
        np.add.at(S, rows, vals)
    pos = (zn * np.roll(zn, -B, axis=0)).sum(axis=1)
    loss = np.log(S) + INV_T - INV_T * pos
    return np.asarray(np.float32(loss.mean()))


def kernel(emb0: np.ndarray, emb1: np.ndarray) -> np.ndarray:
    zn, in_maps = prepare_in_maps(emb0, emb1)
    res = bass_utils.run_bass_kernel_spmd(_get_nc(), in_maps, core_ids=list(range(NCORES)))
    return combine(zn, res.results)


# revision 18
# speedup vs baseline: 2.2158x; 1.0654x over previous
"""Trainium2 Bass kernel for nn_ContrastiveLoss (SimCLR-style, N=8192, D=128).

v4: symmetry + host layout prep.  Global rows form 64 blocks of 128; core c
owns blocks 8c..8c+7.  The host normalizes z, rotates it per core, and ships
the TRANSPOSED bf16 matrix znT [128, 5120] (local col-blocks 0..39), so the
device does zero layout work.  Per local row-block b (strip), the device
computes sim blocks at offsets d=0..32 only (half the matrix, wrap-band):
three chunks of 1536/1536/1152 columns, each matmul'd into a rotating PSUM
tile and exponentiated by ACT (exp(10x-10), bf16 out, fp32 accum_out gives
the row sums).  Transposed contributions (offsets 33..63 of each row) come
from column sums of the d=1..31 blocks: ones[128,32]^T @ E matmuls accumulate
into a packed CS PSUM region (seg s of 512 abs cols -> partition-group slot
via tile_position; bank 6 is drained mid-kernel and recycled for segs 8-9).
Colsums of chunk k are emitted after the fills of chunk k+1 so the in-order
PE queue never stalls on ACT.  Host: scatter colsums to owning rows, add row
sums, subtract diag exp(0)=1, then loss = mean(ln S + 10 - 10*cos_pos).
"""

import sys

sys.path.insert(0, "/opt/trn_rl_repo")

from contextlib import ExitStack

import numpy as np

import concourse.bass as bass
import concourse.bacc as bacc
import concourse.tile as tile
from concourse import mybir
from concourse import bass_utils

B = 4096
D = 128
N = 2 * B
NCORES = 8
ROWS = N // NCORES   # 1024 rows per core
NBLK = 8             # strips per core
NT = 40              # znT col-blocks per core (local blocks 0..39)
ZCOLS = NT * 128     # 5120
SEG = 512
CHUNK = 1536         # chunk 0/1 width; chunk 2 is 1152
INV_T = 10.0
EPS = 1e-8

F32 = mybir.dt.float32
BF16 = mybir.dt.bfloat16
AX = mybir.AxisListType
AF = mybir.ActivationFunctionType


def _cs_slot(seg, life2=False):
    """Colsum psum placement: seg (512 abs cols) -> (bank_col, grp)."""
    if life2:  # segs 8, 9 reuse bank col 0 after the mid-kernel drain
        return 0, seg - 8
    if seg < 4:
        return 0, seg
    return 512, seg - 4


def _chunk_w(dc):
    return CHUNK if dc < 2 else 1152


def _build() -> bass.Bass:
    nc = bacc.Bacc(None)
    zT_in = nc.declare_dram_parameter("zT", [128, ZCOLS], BF16, isOutput=False)
    out_acc = nc.declare_dram_parameter("acc", [128, 3 * NBLK], F32, isOutput=True)
    out_cs = nc.declare_dram_parameter("cs", [4, 1536], F32, isOutput=True)

    with tile.TileContext(nc) as tc:
        with ExitStack() as ctx:
            persist = ctx.enter_context(tc.tile_pool(name="persist", bufs=1))
            epool = ctx.enter_context(tc.tile_pool(name="epool", bufs=3))
            ptpool = ctx.enter_context(tc.tile_pool(name="ptpool", bufs=2, space="PSUM"))
            psB = ctx.enter_context(tc.tile_pool(name="psB", bufs=1, space="PSUM"))

            ones32 = persist.tile([128, 32], BF16)
            nc.vector.memset(ones32, 1.0)
            b_neg10 = persist.tile([128, 1], F32)
            nc.vector.memset(b_neg10, -INV_T)

            znT = persist.tile([128, ZCOLS], BF16)
            acc_sb = persist.tile([128, 3 * NBLK], F32)
            cs_sb = persist.tile([128, 1536], F32)

            CS = psB.tile([128, 1024], F32)       # 2 banks
            nc.vector.memset(CS, 0.0)

            # input DMA in pieces so compute can start early
            for p0, p1 in ((0, 1664), (1664, 3200), (3200, 4352), (4352, ZCOLS)):
                nc.sync.dma_start(out=znT[:, p0:p1], in_=zT_in[:, p0:p1])

            cs_life2 = {"on": False}

            # static last-writer per CS bank-life for stop flags
            chunk_order = [(dc, b) for dc in range(3) for b in range(NBLK)]
            last_of = {}
            for (dc, b) in chunk_order:
                for d in range(max(1, 12 * dc), min(32, 12 * dc + 12)):
                    jb = b + d
                    seg = jb // 4
                    bank = _cs_slot(seg, seg >= 8)[0]
                    last_of[(bank, seg >= 8)] = (dc, b, d)

            def colsum_mms(dc, b, E):
                d = max(1, 12 * dc)
                d_hi = min(32, 12 * dc + 12)
                while d < d_hi:
                    jb = b + d
                    seg = jb // 4
                    life2 = seg >= 8
                    if life2 and not cs_life2["on"]:
                        raise RuntimeError("life2 before drain")
                    bank, grp = _cs_slot(seg, life2)
                    run = 1
                    while (
                        d + run < d_hi
                        and (b + d + run) // 4 == seg
                    ):
                        run += 1
                    stop = any(
                        last_of.get((bank, life2)) == (dc, b, dd)
                        for dd in range(d, d + run)
                    )
                    k = d - 12 * dc
                    off = (jb * 128) % 512
                    nc.tensor.matmul(
                        CS[32 * grp : 32 * grp + 32, bank + off : bank + off + run * 128],
                        ones32,
                        E[:, k * 128 : (k + run) * 128],
                        start=False,
                        stop=stop,
                        tile_position=(0, 32 * grp),
                        skip_group_check=True,
                    )
                    d += run

            pend = None
            for (dc, b) in chunk_order:
                W = _chunk_w(dc)
                pt = ptpool.tile([128, CHUNK], F32, tag="pt", name="pt")
                c0 = b * 128 + dc * CHUNK
                off = 0
                while off < W:
                    w = min(SEG, W - off)
                    nc.tensor.matmul(
                        pt[:, off : off + w],
                        znT[:, b * 128 : (b + 1) * 128],
                        znT[:, c0 + off : c0 + off + w],
                        start=True,
                        stop=True,
                    )
                    off += w
                if pend is not None:
                    if pend[0][:2] == (2, 1):
                        # drain bank 6 (segs 0..3), recycle for segs 8..9;
                        # colsums of chunk (2,1) write seg 8 first
                        nc.vector.tensor_copy(cs_sb[:, 0:512], CS[:, 0:512])
                        nc.vector.memset(CS[:, 0:512], 0.0)
                        cs_life2["on"] = True
                    colsum_mms(*pend[0][:2], pend[1])
                E = epool.tile([128, CHUNK], BF16, tag="E", name="E")
                nc.scalar.activation(
                    E[:, 0:W], pt[:, 0:W], AF.Exp, scale=INV_T, bias=b_neg10,
                    accum_out=acc_sb[:, 3 * b + dc : 3 * b + dc + 1],
                )
                pend = ((dc, b), E)
            if pend[0][:2] == (2, 1):
                raise RuntimeError("unexpected")
            colsum_mms(*pend[0][:2], pend[1])

            nc.vector.tensor_copy(cs_sb[:, 512:1536], CS[:, :])
            nc.sync.dma_start(out=out_acc[:, :], in_=acc_sb)
            # only partition rows 0/32/64/96 of cs_sb carry data
            nc.sync.dma_start(out=out_cs[:, :], in_=cs_sb[0:97:32, :])

    nc.compile()
    return nc


_NC = None


def _get_nc() -> bass.Bass:
    global _NC
    if _NC is None:
        _NC = _build()
    return _NC


def prepare_in_maps(emb0: np.ndarray, emb1: np.ndarray):
    import ml_dtypes

    z = np.concatenate(
        [np.asarray(emb0, np.float32), np.asarray(emb1, np.float32)], axis=0
    )
    nrm = np.maximum(np.linalg.norm(z, axis=1, keepdims=True), EPS)
    zn = (z / nrm).astype(np.float32)
    in_maps = []
    for c in range(NCORES):
        zr = np.roll(zn, -c * ROWS, axis=0)[:ZCOLS]
        zT = np.ascontiguousarray(zr.T).astype(ml_dtypes.bfloat16)
        in_maps.append({"zT": zT})
    return zn, in_maps


def combine(zn: np.ndarray, results) -> np.ndarray:
    S = np.zeros(N, dtype=np.float64)
    for c in range(NCORES):
        acc = np.asarray(results[c]["acc"], np.float64)   # [128, 24]
        cs = np.asarray(results[c]["cs"], np.float64)     # [4, 1536]
        for b in range(NBLK):
            rows = (c * ROWS + b * 128 + np.arange(128)) % N
            S[rows] += acc[:, 3 * b] + acc[:, 3 * b + 1] + acc[:, 3 * b + 2] - 1.0
        j = np.arange(128, 4992)
        seg = j // 512
        off = j % 512
        vals = np.empty(j.shape, np.float64)
        for s in range(10):
            m = seg == s
            if s < 4:
                vals[m] = cs[s, off[m]]                  # drain1: bank6 life1
            elif s < 8:
                vals[m] = cs[s - 4, 1024 + off[m]]       # final: bank7
            else:
                vals[m] = cs[s - 8, 512 + off[m]]        # final: bank6 life2
        rows = (c * ROWS + j) % N
        np.add.at(S, rows, vals)
    pos = (zn * np.roll(zn, -B, axis=0)).sum(axis=1)
    loss = np.log(S) + INV_T - INV_T * pos
    return np.asarray(np.float32(loss.mean()))


def kernel(emb0: np.ndarray, emb1: np.ndarray) -> np.ndarray:
    zn, in_maps = prepare_in_maps(emb0, emb1)
    res = bass_utils.run_bass_kernel_spmd(_get_nc(), in_maps, core_ids=list(range(NCORES)))
    return combine(zn, res.results)
